# revision 111
# baseline (speedup 1.0000x reference)
"""Trainium2 Bass kernel for CSSM (Mamba-style 2D selective scan block).

Sharding: 8 cores = 4 batch x 2 d_inner-halves. Each core computes the
full front-end (convs/projections) for its batch element, the selective
scan for its 96 d_inner channels x 16 states, and a partial output
projection. The host sums the two partial outputs per batch element.

Engine split per 1024-column block: the 16 recurrences run as native
tensor_tensor_scan instructions on the Vector engine (the only engine
the ISA allows them on); dA_n = exp(-(n+1)*delta) comes from the scalar
engine (16 exps per block, emitted at the head of each pipeline
iteration so the scans never wait); the dBu = delta*u*B products run on
GPSIMD, pipelined one block ahead so its lower throughput hides behind
the scan stage; h*C products and the gate run on Vector; y = sum_n C.h
accumulates on the tensor engine via an identity matmul into PSUM.

The 1x1 proj and the 3x3 dconv are fused into one 9-tap conv with
host-composed weights (96-channel contraction); x is DMA'd with a one-
row halo directly into a zero-padded buffer, which removes the proj
pipeline stage entirely.
"""
import sys

sys.path.insert(0, "/opt/trn_rl_repo")

import numpy as np

C = 96            # d_model; also channels per d_inner half
DI = 192          # d_inner
NST = 16          # d_state
DTR = 6           # dt_rank
HH = 64
WW = 64
L = HH * WW       # 4096
T = 512           # matmul moving-dim chunk
T2 = 1024         # scan-phase chunk (2 matmul chunks)
NB = L // T2      # 4 scan-phase blocks
PW = WW + 2       # 66: padded row width for the 3x3 conv
G = 68            # left guard of the padded conv buffer
PADLEN = G + (HH + 2) * PW + 68
BLOCKS = ((0, 1024), (1024, 1024), (2048, 1024), (3072, 1024))
NE_EARLY = 12          # dA exps emitted one iteration early (between the
                      # edt exp and the ln, so no extra table load): the
                      # first scans of a block never wait on the scalar
                      # engine even when it lags a few us
TMP_GP = frozenset()  # h*C products routed to GPSIMD (if any, use last
                      # states only: GPSIMD reaches them after its dBu
                      # batch, so earlier h tiles would pin the pool)

_CACHE = {}


def _emit(tc, nc, mybir, dram):
    from contextlib import ExitStack

    from concourse import bass

    f32 = mybir.dt.float32
    bf16 = mybir.dt.bfloat16
    f16 = mybir.dt.float16
    AF = mybir.ActivationFunctionType
    OP = mybir.AluOpType

    def mmacc(out, pairs, start=True, stop=True, ncols=None):
        """Matmul with free-dim split into even <=512-col PSUM-bank groups."""
        n = ncols if ncols is not None else out.shape[-1]
        nchunk = -(-n // T)
        step = -(-n // nchunk)
        bounds = list(range(0, n, step)) + [n]
        for c0, c1 in zip(bounds[:-1], bounds[1:]):
            for i, (lh, rh) in enumerate(pairs):
                nc.tensor.matmul(out[:, c0:c1], lh, rh[:, c0:c1],
                                 start=start and i == 0,
                                 stop=stop and i == len(pairs) - 1)

    with ExitStack() as ctx:
        ec = ctx.enter_context
        consts = ec(tc.tile_pool(name="consts", bufs=1))
        persist = ec(tc.tile_pool(name="persist", bufs=1))
        dpool = ec(tc.tile_pool(name="dpool", bufs=1, space="DRAM"))
        fw = ec(tc.tile_pool(name="fw", bufs=1))
        pxc2 = ec(tc.tile_pool(name="pxc2", bufs=3))
        pxca = ec(tc.tile_pool(name="pxca", bufs=4))
        pxcb = ec(tc.tile_pool(name="pxcb", bufs=2))
        psz = ec(tc.tile_pool(name="psz", bufs=3))
        pxd = ec(tc.tile_pool(name="pxd", bufs=2))
        dl = ec(tc.tile_pool(name="dl", bufs=3))
        edtp = ec(tc.tile_pool(name="edtp", bufs=1))
        lp = ec(tc.tile_pool(name="lp", bufs=3))
        dap = ec(tc.tile_pool(name="dap", bufs=19))
        dbup = ec(tc.tile_pool(name="dbup", bufs=NST + 2))
        hp = ec(tc.tile_pool(name="hp", bufs=4))
        bcb = ec(tc.tile_pool(name="bcb", bufs=10))
        bcc = ec(tc.tile_pool(name="bcc", bufs=18))
        tl = ec(tc.tile_pool(name="tl", bufs=2))
        fps = ec(tc.tile_pool(name="fps", bufs=2, space="PSUM"))
        psy = ec(tc.tile_pool(name="psy", bufs=2, space="PSUM"))
        pmix = ec(tc.tile_pool(name="pmix", bufs=2, space="PSUM"))

        def cload(name, shape, dtype=f32, rearr=None, pool=None):
            t = (pool or consts).tile(list(shape), dtype, tag=name)
            src = dram[name]
            if rearr is not None:
                src = src.rearrange(rearr)
            nc.sync.dma_start(t[:], src)
            return t

        # dconv weights first: they gate the first front-end matmul
        wd_sb = cload("wd", (C, 9, C), bf16, "t k m -> k t m", pool=fw)

        # zero-padded x image; only the guard cells (never overwritten by
        # the row DMAs, which fill cols 1..64 of rows 1..64) need zeroing.
        xp1 = persist.tile([C, PADLEN], bf16, tag="xp1", name="xp1")
        nc.gpsimd.memset(xp1[:, :G + PW + 1], 0.0)          # guard + top row
        nc.gpsimd.memset(xp1[:, PADLEN - PW - G - 1:], 0.0)  # bottom + guard
        pads = xp1[:, G + PW + 65: G + PW + 65 + (HH - 1) * PW]
        nc.gpsimd.memset(pads.rearrange("p (r w) -> p r w", w=PW)[:, :, 0:2],
                         0.0)                                # inter-row pads

        def _xload(s):
            cs, bw = BLOCKS[s]
            r0 = cs // WW
            nr = min(HH - 1, r0 + bw // WW) - r0 + 1
            dstx = xp1[:, G + (r0 + 1) * PW + 1: G + (r0 + nr + 1) * PW + 1]
            dstx = dstx.rearrange("p (r w) -> p r w", w=PW)[:, :, 0:WW]
            srcx = dram["x"][:, r0 * WW: (r0 + nr) * WW]
            nc.sync.dma_start(dstx, srcx.rearrange("p (r w) -> p r w", w=WW))

        # block 0's rows go right behind the dconv weights, ahead of the
        # non-critical constant loads
        _xload(0)

        # PE p-state warm-up: ~3us of continuous back-to-back matmuls
        # while the x rows stream in, so block 0's front-end runs at full
        # clock instead of the cold 0.65GHz p-state
        warm = fps.tile([C, T2], f32, tag="fps", name="pe_warm")
        for i in range(40):
            nc.tensor.matmul(warm[:, :C], wd_sb[:, 0, :], wd_sb[:, 1, :])

        win_sb = cload("win", (C, C), bf16, pool=fw)
        w1d_sb = cload("w1d", (C, 8, C), bf16, "g t k m -> k (g t) m", pool=fw)
        wxp_sb = cload("wxp", (C, 2, 38), bf16, "g k m -> k g m", pool=fw)
        wdt_sb = cload("wdt", (32, C), bf16)
        bdt_sb = cload("bdt", (C, 1))
        wdd_sb = cload("wdd", (C, C), bf16)
        wout_sb = cload("wout", (C, C), bf16)
        b1d_sb = cload("b1d", (C, 2))
        aneg_sb = cload("aneg", (C, NST))       # -exp(A_log), local rows
        ident_sb = cload("ident", (C, C), bf16)

        carry = persist.tile([C, NST], f32, tag="carry")
        xdd = dpool.tile([38, L], bf16, tag="xdd")

        state = {}

        def _front(s):
            cs, bw = BLOCKS[s]
            ce = cs + bw
            nhh = bw // T
            nrow = bw // WW
            r0 = cs // WW

            # ---- fused 1x1-proj + 3x3 dconv (9 taps, 96-contraction) ----
            units = []
            ro = 0
            while ro < nrow:
                units.append((ro, min(7, nrow - ro)))
                ro += 7
            xc2 = pxc2.tile([C, 3 + T2], bf16, tag="xc2",
                            name=f"xc2_{s}")[:, :3 + bw]
            if s == 0:
                nc.gpsimd.memset(xc2[:, 0:3], 0.0)
            else:
                pw = BLOCKS[s - 1][1]
                nc.gpsimd.tensor_copy(xc2[:, 0:3],
                                      state["xc2_prev"][:, pw:pw + 3])
            for u, (ro, rows) in enumerate(units):
                ru = r0 + ro
                cols = rows * PW
                base = G + (ru + 1) * PW
                psd = fps.tile([C, T2], f32, tag="fps", name=f"dconv_{s}_{u}")
                pairs = []
                for tap in range(9):
                    dy, dx = tap // 3, tap % 3
                    shift = (dy - 1) * PW + (dx - 1)
                    pairs.append((wd_sb[:, tap, :],
                                  xp1[:, base + shift: base + shift + cols]))
                mmacc(psd[:, :cols], pairs, ncols=cols)
                srcv = psd[:, :cols].rearrange("p (r w) -> p r w", w=PW)[:, :, 1:65]
                dstv = xc2[:, 3 + ro * WW: 3 + (ro + rows) * WW]
                dstv = dstv.rearrange("p (r w) -> p r w", w=WW)
                if s <= 1:
                    # fill: Vector is under-loaded; keep the scalar
                    # engine's serial chain to delta as short as possible
                    nc.vector.tensor_copy(dstv, srcv)
                else:
                    nc.scalar.activation(dstv, srcv, AF.Copy)
            state["xc2_prev"] = xc2

            # ---- in_proj (+ fused causal conv1d) + silus ----
            # conv1d composed into in_proj on the host: xc_g[t] =
            # sum_k (diag(w1_k) W_in_g) . xc2[t-3+k], so the xin tiles
            # and their PSUM->SBUF copies disappear entirely.
            sz = psz.tile([C, T2], bf16, tag="sz", name=f"sz_{s}")[:, :bw]
            xc_a = pxca.tile([C, T2], bf16, tag="xc_a", name=f"xc_a_{s}")[:, :bw]
            xc_b = pxcb.tile([C, T2], bf16, tag="xc_b", name=f"xc_b_{s}")[:, :bw]
            psz_ = fps.tile([C, T2], f32, tag="fps", name=f"inp_{s}_z")[:, :bw]
            mmacc(psz_, [(win_sb[:], xc2[:, 3:3 + bw])])
            nc.scalar.activation(sz[:], psz_[:], AF.Silu)
            for g in range(2):
                ps = fps.tile([C, T2], f32, tag="fps", name=f"c1d_{s}_{g}")[:, :bw]
                mmacc(ps, [(w1d_sb[:, g * 4 + k, :], xc2[:, k:k + bw])
                           for k in range(4)])
                nc.scalar.activation(xc_a[:] if g == 0 else xc_b[:], ps[:],
                                     AF.Silu, bias=b1d_sb[:, g:g + 1])

            # ---- x_proj -> x_dbl block, staged to DRAM ----
            psx = fps.tile([38, T2], f32, tag="fps", name=f"xp_{s}")[:, :bw]
            mmacc(psx, [(wxp_sb[:, 0, :], xc_a[:]), (wxp_sb[:, 1, :], xc_b[:])])
            x_dbl = pxd.tile([38, T2], bf16, tag="x_dbl", name=f"x_dbl_{s}")[:, :bw]
            if s <= 1:
                nc.vector.tensor_copy(x_dbl[:], psx[:])
            else:
                nc.scalar.activation(x_dbl[:], psx[:], AF.Copy)
            nc.sync.dma_start(xdd[:, cs:ce], x_dbl[:])

            # ---- delta = softplus(dt_proj): exp here, ln1p in _ln ----
            edt = edtp.tile([C, T2], f32, tag="edt", name=f"edt_{s}")[:, :bw]
            for hh in range(nhh):
                psD = pmix.tile([C, T], f32, tag="pmix", name=f"psD_{s}_{hh}")
                nc.tensor.matmul(psD[:], wdt_sb[:],
                                 x_dbl[:32, hh * T:(hh + 1) * T])
                nc.scalar.activation(edt[:, hh * T:(hh + 1) * T], psD[:],
                                     AF.Exp, bias=bdt_sb[:])

            state[("xca", s)] = xc_a
            state[("sz", s)] = sz
            state[("edt", s)] = edt

        def _ln(s):
            cs, bw = BLOCKS[s]
            edt = state.pop(("edt", s))
            delta_c = dl.tile([C, T2], f16, tag="delta", name=f"delta_{s}")[:, :bw]
            nc.scalar.activation(delta_c[:], edt[:], AF.Ln, bias=1.0)
            state[("delta", s)] = delta_c

        def _exp_early(s):
            # first NE_EARLY dA exps, one iteration ahead of their scans;
            # emitted while the exp table is already resident
            cs, bw = BLOCKS[s]
            delta_c = state[("delta", s)]
            dAs = []
            for n in range(NE_EARLY):
                dA = dap.tile([C, T2], f16, tag="dA",
                              name=f"dA_{s}_{n}")[:, :bw]
                nc.scalar.activation(dA[:], delta_c[:], AF.Exp,
                                     scale=aneg_sb[:, n:n + 1])
                dAs.append(dA)
            state[("dAs", s)] = dAs

        def _bcast(row, tile_):
            nc.sync.dma_start(tile_[:], bass.AP(
                tensor=row.tensor, offset=row.offset,
                ap=[[0, C], [1, tile_.shape[-1]]]))

        def _dma_pro(sf, sb, sc):
            # Iteration prologue on the single in-order DMA queue: the x
            # rows for this iteration's front-end first (they gate the
            # whole dconv->delta chain), then the B rows for the dBu
            # stage (block sb) interleaved with the C rows for the scan
            # stage (block sc) so neither consumer starves.
            if sf is not None:
                _xload(sf)
            bcs = [] if sb is None else state.setdefault(("bcs", sb), [])
            ccs = [] if sc is None else state.setdefault(("ccs", sc), [])
            for n in range(NST):
                if sb is not None:
                    cs, bw = BLOCKS[sb]
                    tb = bcb.tile([C, T2], bf16, tag="bcB",
                                  name=f"bcB_{sb}_{n}")[:, :bw]
                    _bcast(xdd[DTR + n: DTR + n + 1, cs:cs + bw], tb)
                    bcs.append(tb)
                if sc is not None:
                    cs, bw = BLOCKS[sc]
                    tc_ = bcc.tile([C, T2], bf16, tag="bcC",
                                   name=f"bcC_{sc}_{n}")[:, :bw]
                    _bcast(xdd[DTR + NST + n: DTR + NST + n + 1, cs:cs + bw],
                           tc_)
                    ccs.append(tc_)

        def _bprep(s):
            # One iteration ahead of the scan: du and the dBu products
            # (GPSIMD) for block s. Everything this stage reads is at
            # least one iteration old, so GPSIMD chews through its
            # ~2.1us/state pipe undisturbed. Block 0 runs entirely on
            # Vector (fill: everything else is idle).
            cs, bw = BLOCKS[s]
            du_c = state[("du", s)]
            bcs = state.pop(("bcs", s))
            dbus = {}
            for n in range(NST):
                dBu = dbup.tile([C, T2], f16, tag="dBu",
                                name=f"dBu_{s}_{n}")[:, :bw]
                # du is a full iteration old, so the two head products on
                # Vector are stall-free and cover GPSIMD's ramp
                if s == 0 or n < 2:
                    nc.vector.tensor_mul(dBu[:], du_c[:], bcs[n][:])
                else:
                    nc.gpsimd.tensor_mul(dBu[:], du_c[:], bcs[n][:])
                dbus[n] = dBu
            state[("dbus", s)] = dbus

        def _bprep_tail(sd, su):
            # du for block su at the Vector queue's tail: its delta has
            # just landed on the scalar engine by the time Vector gets
            # here, and its consumers run next iteration.
            if su is not None:
                cs, bw = BLOCKS[su]
                du_c = dl.tile([C, T2], f16, tag="du", name=f"du_{su}")[:, :bw]
                nc.vector.tensor_mul(du_c[:], state[("delta", su)][:],
                                     state[("xca", su)][:])
                state[("du", su)] = du_c

        def _scan_pre(s):
            # remaining dA exps at the head of the iteration (the scalar
            # engine still holds the exp table from the previous
            # iteration's tail)
            cs, bw = BLOCKS[s]
            delta_c = state.pop(("delta", s))
            dAs = state[("dAs", s)]
            for n in range(NE_EARLY, NST):
                dA = dap.tile([C, T2], f16, tag="dA",
                              name=f"dA_{s}_{n}")[:, :bw]
                nc.scalar.activation(dA[:], delta_c[:], AF.Exp,
                                     scale=aneg_sb[:, n:n + 1])
                dAs.append(dA)

        def _scan(s):
            cs, bw = BLOCKS[s]
            nhh = bw // T
            xc_a = state.pop(("xca", s))
            sz = state.pop(("sz", s))
            ccs = state.pop(("ccs", s))
            dbus = state.pop(("dbus", s))
            dAs = state.pop(("dAs", s))

            yPh = [psy.tile([C, T], f32, tag="psy", name=f"yP_{s}_{hh}")
                   for hh in range(nhh)]

            def emit_tmp(n, h):
                tmp = lp.tile([C, T2], bf16, tag="tmp",
                              name=f"tmp_{s}_{n}")[:, :bw]
                if n in TMP_GP and s > 0:
                    nc.gpsimd.scalar_tensor_tensor(tmp[:], h[:], 1.0,
                                                   ccs[n][:], OP.mult, OP.mult)
                else:
                    nc.vector.tensor_mul(tmp[:], h[:], ccs[n][:])
                for hh in range(nhh):
                    nc.tensor.matmul(yPh[hh][:], ident_sb[:],
                                     tmp[:, hh * T:(hh + 1) * T],
                                     start=(n == 0), stop=False)

            # tmp is emitted 2 states late so the vector engine never
            # head-of-line blocks on a scan that just retired.
            pend = []
            for n in range(NST):
                h = hp.tile([C, T2], f16, tag="h", name=f"h_{s}_{n}")[:, :bw]
                init = 0.0 if s == 0 else carry[:, n:n + 1]
                nc.vector.tensor_tensor_scan(h[:], dAs[n][:], dbus[n][:],
                                             init, OP.mult, OP.add)
                nc.vector.tensor_copy(carry[:, n:n + 1], h[:, bw - 1:bw])
                pend.append((n, h))
                if len(pend) > 2:
                    emit_tmp(*pend.pop(0))
            for item in pend:
                emit_tmp(*item)

            # ---- D*u folded into the accumulator, gate, out_proj ----
            for hh in range(nhh):
                sl = slice(hh * T, (hh + 1) * T)
                nc.tensor.matmul(yPh[hh][:], wdd_sb[:], xc_a[:, sl],
                                 start=False, stop=True)
                y2 = tl.tile([C, T], bf16, tag="y2", name=f"y2_{s}_{hh}")
                nc.vector.tensor_mul(y2[:], yPh[hh][:], sz[:, sl])
                outP = pmix.tile([C, T], f32, tag="pmix", name=f"outP_{s}_{hh}")
                nc.tensor.matmul(outP[:], wout_sb[:], y2[:])
                osb = tl.tile([C, T], f32, tag="osb", name=f"osb_{s}_{hh}")
                nc.scalar.activation(osb[:], outP[:], AF.Copy)
                nc.sync.dma_start(
                    dram["out_part"][:, cs + hh * T: cs + (hh + 1) * T], osb[:])

        # Depth-3 software pipeline: front-end at block b, dBu prep at
        # b-1, scan at b-2 — every stage's inputs are >=1 iteration old.
        NBK = len(BLOCKS)
        for b in range(NBK + 2):
            _dma_pro(b if 0 < b < NBK else None,
                     b - 1 if 1 <= b <= NBK else None,
                     b - 2 if b >= 2 else None)
            if b >= 2:
                _scan_pre(b - 2)
            if 1 <= b <= NBK:
                _bprep(b - 1)
            if b < NBK:
                _front(b)
            if 1 <= b <= NBK:
                _exp_early(b - 1)
            if b < NBK:
                _ln(b)
            if b >= 2:
                _scan(b - 2)
            _bprep_tail(b - 1 if 1 <= b <= NBK else None,
                        b if b < NBK else None)


def _build_program():
    from concourse import bacc, tile, mybir

    nc = bacc.Bacc("TRN2", target_bir_lowering=False, debug=False, num_devices=8)
    f32 = mybir.dt.float32
    bf16 = mybir.dt.bfloat16

    def din(name, shape, dtype=f32):
        return nc.dram_tensor(name, shape, dtype, kind="ExternalInput").ap()

    dram = {
        "x": din("x", (C, L), bf16),
        "wd": din("wd", (9, C, C), bf16),
        "win": din("win", (C, C), bf16),
        "w1d": din("w1d", (2, 4, C, C), bf16),
        "b1d": din("b1d", (C, 2)),
        "wxp": din("wxp", (2, C, 38), bf16),
        "wdt": din("wdt", (32, C), bf16),
        "bdt": din("bdt", (C, 1)),
        "wdd": din("wdd", (C, C), bf16),
        "wout": din("wout", (C, C), bf16),
        "aneg": din("aneg", (C, NST)),
        "ident": din("ident", (C, C), bf16),
        "out_part": nc.dram_tensor("out_part", (C, L), f32,
                                   kind="ExternalOutput").ap(),
    }

    with tile.TileContext(nc) as tc:
        _emit(tc, nc, mybir, dram)
    nc.compile()
    return nc


def get_program():
    if "nc" not in _CACHE:
        _CACHE["nc"] = _build_program()
    return _CACHE["nc"]


def make_core_inputs(inputs, b, half):
    import ml_dtypes

    bf = ml_dtypes.bfloat16
    perm = np.concatenate([
        np.arange(half * C, half * C + C),
        np.arange((1 - half) * C, (1 - half) * C + C),
    ])
    loc = perm[:C]

    a = np.exp(np.asarray(inputs["A_log"], np.float64))[loc].astype(np.float32)

    # fused 1x1-proj + 3x3-dconv weights: W[tap][i][o] = sum_m dconv[o,m]*proj[m,i]
    dw = np.asarray(inputs["dconv_w"], np.float64)       # (96, 192, 3, 3)
    pw = np.asarray(inputs["proj_w"], np.float64)[:, :, 0, 0]  # (192, 96)
    wd = np.einsum("omyx,mi->yxio", dw, pw).reshape(9, C, C).astype(np.float32)

    # conv1d composed into in_proj: lhs[g,k][c,d] = W_in[ch_d, c] * w1[ch_d, k]
    w_in = np.asarray(inputs["in_proj_w"], np.float64)
    win = w_in[DI + loc].T.astype(np.float32)               # z group only
    w1 = np.asarray(inputs["conv1d_w"], np.float64)[perm]   # (192, 4)
    w1d = np.zeros((2, 4, C, C), np.float32)
    for g in range(2):
        wing = w_in[perm[g * C:(g + 1) * C]]                # (d, c)
        for k in range(4):
            w1d[g, k] = (wing * w1[g * C:(g + 1) * C, k][:, None]).T
    b1 = np.asarray(inputs["conv1d_b"], np.float32)[perm]
    b1d = np.stack([b1[:C], b1[C:]], axis=1)

    wxp_full = np.asarray(inputs["x_proj_w"], np.float32)[:, perm]  # (38, 192)
    wxp = np.stack([wxp_full[:, :C].T, wxp_full[:, C:].T], axis=0)

    wdt = np.zeros((32, C), np.float32)
    wdt[:DTR] = np.asarray(inputs["dt_proj_w"], np.float32)[loc].T

    return {
        "x": np.ascontiguousarray(
            np.asarray(inputs["x"], np.float32)[b].reshape(C, L)).astype(bf),
        "wd": wd.astype(bf),
        "win": np.ascontiguousarray(win).astype(bf),
        "w1d": w1d.astype(bf),
        "b1d": np.ascontiguousarray(b1d),
        "wxp": np.ascontiguousarray(wxp).astype(bf),
        "wdt": wdt.astype(bf),
        "bdt": np.asarray(inputs["dt_proj_b"], np.float32)[loc, None],
        "wdd": np.diag(np.asarray(inputs["D"], np.float32)[loc]).astype(bf),
        "wout": np.ascontiguousarray(
            np.asarray(inputs["out_proj_w"], np.float32)[:, loc].T).astype(bf),
        "aneg": -a,
        "ident": np.eye(C, dtype=np.float32).astype(bf),
    }


def kernel(**inputs):
    from concourse import bass_utils

    nc = get_program()
    in_maps = [make_core_inputs(inputs, b, half)
               for b in range(4) for half in range(2)]
    res = bass_utils.run_bass_kernel_spmd(nc, in_maps, core_ids=list(range(8)))
    out = np.zeros((4, C, L), np.float32)
    for b in range(4):
        out[b] = res.results[2 * b]["out_part"] + res.results[2 * b + 1]["out_part"]
    return out.reshape(4, C, HH, WW)
